# revision 6
# baseline (speedup 1.0000x reference)
"""Trainium2 Bass kernel for a pre-norm transformer encoder layer (SwiGLU FFN).

Shapes (hardcoded): x [2, 2048, 768], mask [2, 2048, 2048] int32,
wq/wk/wv/wo [768, 768], w1/w3 [3072, 768], w2 [768, 3072], g_attn/g_ffn [768].

Sharding: 8 cores = 2 batch x 4 query-slices of 512 tokens. Each core
computes K/V for its full batch element (replicated within the group of 4)
and attention + FFN for its own 512 tokens. No collectives.

On-device layout is feature-major ("transposed"): activations [D, tokens].
All matmuls run in bf16 with fp32 PSUM accumulation.
"""
import os
import sys

for _p in ("/opt/trn_rl_repo", "/root/.axon_site/_ro/trn_rl_repo"):
    if os.path.isdir(_p) and _p not in sys.path:
        sys.path.append(_p)

import numpy as np
import ml_dtypes

import concourse.bacc as bacc
import concourse.tile as tile
from concourse import mybir

F32 = mybir.dt.float32
BF16 = mybir.dt.bfloat16
AF = mybir.ActivationFunctionType

B, S, D, H = 2, 2048, 768, 12
DK = D // H            # 64
F = 4 * D              # 3072
T = 512                # local query tokens per core
NCH = D // 128         # 6 feature chunks
NFC = F // 128         # 24 FFN chunks
NKT = S // 128         # 16 key tiles
NQT = S // T           # 4 query slices per batch element
EPS = 1e-5


def build_nc():
    nc = bacc.Bacc("TRN2", target_bir_lowering=False, debug=False, num_devices=8)

    xT = nc.dram_tensor("xT", [NCH, 128, S], F32, kind="ExternalInput").ap()
    maskT = nc.dram_tensor("maskT", [128, NKT * T], BF16, kind="ExternalInput").ap()
    wqT = nc.dram_tensor("wqT", [NCH, 128, D], BF16, kind="ExternalInput").ap()
    wkT = nc.dram_tensor("wkT", [NCH, 128, D], BF16, kind="ExternalInput").ap()
    wvT = nc.dram_tensor("wvT", [NCH, 128, D], BF16, kind="ExternalInput").ap()
    woT = nc.dram_tensor("woT", [H, DK, D], BF16, kind="ExternalInput").ap()
    w1T = nc.dram_tensor("w1T", [NFC, NCH, 128, 128], BF16, kind="ExternalInput").ap()
    w3T = nc.dram_tensor("w3T", [NFC, NCH, 128, 128], BF16, kind="ExternalInput").ap()
    w2T = nc.dram_tensor("w2T", [NCH, NFC, 128, 128], BF16, kind="ExternalInput").ap()
    ones16 = nc.dram_tensor("ones16", [128, 128], BF16, kind="ExternalInput").ap()
    onesf = nc.dram_tensor("onesf", [1, DK], F32, kind="ExternalInput").ap()

    outT = nc.dram_tensor("outT", [NCH, 128, T], F32, kind="ExternalOutput").ap()

    with tile.TileContext(nc) as tc:
        with tc.tile_pool(name="glob", bufs=1) as Pg:
            ones16_t = Pg.tile([128, 128], BF16, name="ones16_t")
            onesf_t = Pg.tile([1, DK], F32, name="onesf_t")
            nc.sync.dma_start(ones16_t[:], ones16)
            nc.sync.dma_start(onesf_t[:], onesf)
            eps_t = Pg.tile([128, 1], F32, name="eps_t")
            nc.vector.memset(eps_t[:], EPS)
            xloc = [Pg.tile([128, T], F32, name=f"xloc{c}") for c in range(NCH)]
            hT = [Pg.tile([128, T], F32, name=f"hT{c}") for c in range(NCH)]

            with tc.tile_pool(name="attn", bufs=1) as Pa:
                KT = [Pa.tile([128, S], BF16, name=f"KT{c}") for c in range(NCH)]
                QT = [Pa.tile([128, T], BF16, name=f"QT{c}") for c in range(NCH)]
                VA = [Pa.tile([128, H * (DK + 1)], BF16, name=f"VA{t}")
                      for t in range(NKT)]
                maskT_t = Pa.tile([128, NKT * T], BF16, name="maskT_t")
                nc.sync.dma_start(maskT_t[:], maskT)
                attnT = [Pa.tile([DK, T], BF16, name=f"attnT{h}") for h in range(H)]

                # ---------------- stage 1: rmsnorm + Q/K/V projections --------
                with (
                    tc.tile_pool(name="s1", bufs=1) as P1,
                    tc.tile_pool(name="ps1", bufs=1, space="PSUM") as PS1,
                ):
                    wq_t = [P1.tile([128, D], BF16, name=f"wq{c}") for c in range(NCH)]
                    wk_t = [P1.tile([128, D], BF16, name=f"wk{c}") for c in range(NCH)]
                    wv_t = [P1.tile([128, D], BF16, name=f"wv{c}") for c in range(NCH)]
                    for c in range(NCH):
                        nc.sync.dma_start(wq_t[c][:], wqT[c])
                        nc.sync.dma_start(wk_t[c][:], wkT[c])
                        nc.sync.dma_start(wv_t[c][:], wvT[c])

                    for qt in range(NQT):
                        sl = slice(qt * T, (qt + 1) * T)
                        xq = [P1.tile([128, T], F32, name=f"xq{c}", tag=f"xq{c}")
                              for c in range(NCH)]
                        for c in range(NCH):
                            nc.sync.dma_start(xq[c][:], xT[c][:, sl])
                        # mean(x^2) replicated over partitions via ones-matmul
                        ps_ms = PS1.tile([128, T], F32, tag="ps_ms", name="ps_ms")
                        for c in range(NCH):
                            sq = P1.tile([128, T], BF16, tag="sq", bufs=2,
                                         name=f"sq{qt}_{c}")
                            nc.scalar.activation(sq[:], xq[c][:], AF.Square)
                            nc.tensor.matmul(ps_ms[:], ones16_t[:], sq[:],
                                             start=(c == 0), stop=(c == NCH - 1))
                        lntmp = P1.tile([128, T], F32, tag="lntmp", bufs=2,
                                        name=f"ln{qt}")
                        nc.scalar.activation(lntmp[:], ps_ms[:], AF.Ln,
                                             bias=eps_t[:], scale=1.0 / D)
                        rstd = P1.tile([128, T], F32, tag="rstd", bufs=2,
                                       name=f"rstd{qt}")
                        nc.scalar.activation(rstd[:], lntmp[:], AF.Exp, scale=-0.5)
                        xn = [P1.tile([128, T], BF16, name=f"xn{c}", tag=f"xn{c}",
                                      bufs=2) for c in range(NCH)]
                        for c in range(NCH):
                            nc.vector.tensor_mul(xn[c][:], xq[c][:], rstd[:])

                        local = (qt == QT_IDX)
                        if local:
                            for c in range(NCH):
                                nc.vector.tensor_copy(xloc[c][:], xq[c][:])
                        # K projection (and Q for the local slice)
                        for do in range(NCH):
                            ps_k = PS1.tile([128, T], F32, tag="ps_k", bufs=2,
                                            name=f"ps_k{qt}_{do}")
                            for c in range(NCH):
                                nc.tensor.matmul(
                                    ps_k[:], wk_t[c][:, do * 128:(do + 1) * 128],
                                    xn[c][:], start=(c == 0), stop=(c == NCH - 1))
                            nc.scalar.copy(KT[do][:, sl], ps_k[:])
                            if local:
                                ps_q = PS1.tile([128, T], F32, tag="ps_q",
                                                name=f"ps_q{do}")
                                for c in range(NCH):
                                    nc.tensor.matmul(
                                        ps_q[:], wq_t[c][:, do * 128:(do + 1) * 128],
                                        xn[c][:], start=(c == 0), stop=(c == NCH - 1))
                                nc.scalar.mul(QT[do][:], ps_q[:], 1.0 / np.sqrt(DK))
                        # V projection: token-major tiles with ones columns
                        for tt in range(4):
                            gt = qt * 4 + tt
                            ps_v = PS1.tile([128, D], F32, tag="ps_v", bufs=2,
                                            name=f"ps_v{gt}")
                            tsl = slice(tt * 128, (tt + 1) * 128)
                            for c in range(NCH):
                                nc.tensor.matmul(
                                    ps_v[:, 0:512], xn[c][:, tsl],
                                    wv_t[c][:, 0:512],
                                    start=(c == 0), stop=(c == NCH - 1))
                                nc.tensor.matmul(
                                    ps_v[:, 512:768], xn[c][:, tsl],
                                    wv_t[c][:, 512:768],
                                    start=(c == 0), stop=(c == NCH - 1))
                            nc.vector.memset(VA[gt][:], 1.0)
                            nc.vector.tensor_copy(
                                VA[gt][:].rearrange("p (h e) -> p h e",
                                                    e=DK + 1)[:, :, 0:DK],
                                ps_v[:].rearrange("p (h d) -> p h d", d=DK))

                # ---------------- stage 2: attention ------------------------
                with (
                    tc.tile_pool(name="s2", bufs=1) as P2,
                    tc.tile_pool(name="ps2", bufs=1, space="PSUM") as PS2,
                ):
                    for h in range(H):
                        cc, r0 = h // 2, (h % 2) * DK
                        acc = PS2.tile([128, T], F32, tag="acc", bufs=2,
                                       name=f"acc{h}")
                        for g in range(8):
                            ps_sc = PS2.tile([128, 1024], F32, tag="ps_sc", bufs=2,
                                             name=f"ps_sc{h}_{g}")
                            for j in range(2):
                                kt = 2 * g + j
                                nc.tensor.matmul(
                                    ps_sc[:, j * T:(j + 1) * T],
                                    KT[cc][r0:r0 + DK, kt * 128:(kt + 1) * 128],
                                    QT[cc][r0:r0 + DK, :],
                                    start=True, stop=True)
                            probs = P2.tile([128, 1024], BF16, tag="probs", bufs=3,
                                            name=f"probs{h}_{g}")
                            nc.scalar.activation(probs[:], ps_sc[:], AF.Exp)
                            nc.vector.tensor_mul(
                                probs[:], probs[:],
                                maskT_t[:, g * 1024:(g + 1) * 1024])
                            for j in range(2):
                                kt = 2 * g + j
                                nc.tensor.matmul(
                                    acc[0:DK + 1, :],
                                    VA[kt][:, h * (DK + 1):(h + 1) * (DK + 1)],
                                    probs[:, j * T:(j + 1) * T],
                                    start=(g == 0 and j == 0),
                                    stop=(g == 7 and j == 1))
                        # normalize: recip of row sums, broadcast over DK rows
                        lnrow = P2.tile([1, T], F32, tag="lnrow", bufs=2,
                                        name=f"lnrow{h}")
                        nc.scalar.activation(lnrow[:], acc[DK:DK + 1, :], AF.Ln)
                        srow = P2.tile([1, T], F32, tag="srow", bufs=2,
                                       name=f"srow{h}")
                        nc.scalar.activation(srow[:], lnrow[:], AF.Exp, scale=-1.0)
                        ps_bc = PS2.tile([DK, T], F32, tag="ps_bc", bufs=2,
                                         name=f"ps_bc{h}")
                        nc.tensor.matmul(ps_bc[:], onesf_t[:], srow[:],
                                         start=True, stop=True)
                        bc_sb = P2.tile([DK, T], F32, tag="bc_sb", bufs=2,
                                        name=f"bc_sb{h}")
                        nc.vector.tensor_copy(bc_sb[:], ps_bc[:])
                        nc.vector.tensor_mul(attnT[h][:], acc[0:DK, :], bc_sb[:])

                # ---------------- stage 3: wo projection + residual ---------
                with (
                    tc.tile_pool(name="s3", bufs=1) as P3,
                    tc.tile_pool(name="ps3", bufs=1, space="PSUM") as PS3,
                ):
                    wo_t = [P3.tile([DK, D], BF16, name=f"wo{h}") for h in range(H)]
                    for h in range(H):
                        nc.sync.dma_start(wo_t[h][:], woT[h])
                    for do in range(NCH):
                        ps_h = PS3.tile([128, T], F32, tag="ps_h", bufs=2,
                                        name=f"ps_h{do}")
                        for h in range(H):
                            nc.tensor.matmul(
                                ps_h[:], wo_t[h][:, do * 128:(do + 1) * 128],
                                attnT[h][:], start=(h == 0), stop=(h == H - 1))
                        nc.vector.tensor_add(hT[do][:], ps_h[:], xloc[do][:])

            # ---------------- stage 4: FFN (SwiGLU) -------------------------
            with (
                tc.tile_pool(name="s4", bufs=1) as P4,
                tc.tile_pool(name="ps4", bufs=1, space="PSUM") as PS4,
            ):
                ps_ms2 = PS4.tile([128, T], F32, tag="ps_ms2", name="ps_ms2")
                for c in range(NCH):
                    sqh = P4.tile([128, T], BF16, tag="sqh", bufs=2, name=f"sqh{c}")
                    nc.scalar.activation(sqh[:], hT[c][:], AF.Square)
                    nc.tensor.matmul(ps_ms2[:], ones16_t[:], sqh[:],
                                     start=(c == 0), stop=(c == NCH - 1))
                lntmp2 = P4.tile([128, T], F32, name="lntmp2")
                nc.scalar.activation(lntmp2[:], ps_ms2[:], AF.Ln,
                                     bias=eps_t[:], scale=1.0 / D)
                rstd2 = P4.tile([128, T], F32, name="rstd2")
                nc.scalar.activation(rstd2[:], lntmp2[:], AF.Exp, scale=-0.5)
                hnT = [P4.tile([128, T], BF16, name=f"hnT{c}") for c in range(NCH)]
                for c in range(NCH):
                    nc.vector.tensor_mul(hnT[c][:], hT[c][:], rstd2[:])

                prod = [P4.tile([128, T], BF16, name=f"prod{f}") for f in range(NFC)]
                for f in range(NFC):
                    w1_t = P4.tile([128, D], BF16, tag="w1_t", bufs=2,
                                   name=f"w1_{f}")
                    nc.sync.dma_start(
                        w1_t[:].rearrange("p (c j) -> p c j", j=128),
                        w1T[f].rearrange("c p j -> p c j"))
                    w3_t = P4.tile([128, D], BF16, tag="w3_t", bufs=2,
                                   name=f"w3_{f}")
                    nc.sync.dma_start(
                        w3_t[:].rearrange("p (c j) -> p c j", j=128),
                        w3T[f].rearrange("c p j -> p c j"))
                    ps_u = PS4.tile([128, T], F32, tag="ps_u", bufs=2,
                                    name=f"ps_u{f}")
                    ps_w = PS4.tile([128, T], F32, tag="ps_w", bufs=2,
                                    name=f"ps_w{f}")
                    for c in range(NCH):
                        csl = slice(c * 128, (c + 1) * 128)
                        nc.tensor.matmul(ps_u[:], w1_t[:, csl], hnT[c][:],
                                         start=(c == 0), stop=(c == NCH - 1))
                        nc.tensor.matmul(ps_w[:], w3_t[:, csl], hnT[c][:],
                                         start=(c == 0), stop=(c == NCH - 1))
                    silu = P4.tile([128, T], BF16, tag="silu", bufs=2,
                                   name=f"silu{f}")
                    if os.environ.get("BASS_SIM_SILU") == "1":
                        # CoreSim has no Silu; emulate as u*sigmoid(u)
                        nc.scalar.activation(silu[:], ps_u[:], AF.Sigmoid)
                        nc.vector.tensor_mul(silu[:], silu[:], ps_u[:])
                    else:
                        nc.scalar.activation(silu[:], ps_u[:], AF.Silu)
                    nc.vector.tensor_mul(prod[f][:], silu[:], ps_w[:])

                for do in range(NCH):
                    w2_t = P4.tile([128, F], BF16, tag="w2_t", bufs=2,
                                   name=f"w2_{do}")
                    nc.sync.dma_start(
                        w2_t[:].rearrange("p (f j) -> p f j", j=128),
                        w2T[do].rearrange("f p j -> p f j"))
                    ps_y = PS4.tile([128, T], F32, tag="ps_y", bufs=2,
                                    name=f"ps_y{do}")
                    for f in range(NFC):
                        fsl = slice(f * 128, (f + 1) * 128)
                        nc.tensor.matmul(ps_y[:], w2_t[:, fsl], prod[f][:],
                                         start=(f == 0), stop=(f == NFC - 1))
                    outt = P4.tile([128, T], F32, tag="outt", bufs=2,
                                   name=f"outt{do}")
                    nc.vector.tensor_add(outt[:], ps_y[:], hT[do][:])
                    nc.sync.dma_start(outT[do], outt[:])

    nc.compile()
    return nc


# QT_IDX is the local query-slice index within the batch element. The program
# references it at build time; all cores share one NEFF, so it must be a
# compile-time constant -- we build one NEFF per slice index would be wasteful.
# Instead we make the program identical across cores by noting that the only
# per-core difference stage 1 uses is WHICH quarter is local. To keep a single
# NEFF, the host rotates the token axis per core so that the local slice is
# always quarter 0 (see kernel()).
QT_IDX = 0


def prep_inputs(x, mask, wq, wk, wv, wo, w1, w2, w3, g_attn, g_ffn):
    """Build the 8 per-core input maps (host-side sharding + layout)."""
    bf = ml_dtypes.bfloat16
    wqTe = np.ascontiguousarray((wq * g_attn[None, :]).T.reshape(NCH, 128, D)).astype(bf)
    wkTe = np.ascontiguousarray((wk * g_attn[None, :]).T.reshape(NCH, 128, D)).astype(bf)
    wvTe = np.ascontiguousarray((wv * g_attn[None, :]).T.reshape(NCH, 128, D)).astype(bf)
    woTe = np.ascontiguousarray(wo.T.reshape(H, DK, D)).astype(bf)
    w1Te = np.ascontiguousarray(
        (w1 * g_ffn[None, :]).T.reshape(NCH, 128, NFC, 128)
        .transpose(2, 0, 1, 3)).astype(bf)
    w3Te = np.ascontiguousarray(
        (w3 * g_ffn[None, :]).T.reshape(NCH, 128, NFC, 128)
        .transpose(2, 0, 1, 3)).astype(bf)
    w2Te = np.ascontiguousarray(
        w2.T.reshape(NFC, 128, NCH, 128).transpose(2, 0, 1, 3)).astype(bf)
    ones16 = np.ones((128, 128), bf)
    onesf = np.ones((1, DK), np.float32)

    in_maps = []
    for core in range(8):
        b, qt = core // NQT, core % NQT
        # rotate tokens so the local 512-query slice is always quarter 0
        order = (np.arange(S) + qt * T) % S
        xb = x[b][order]                       # [S, D] rotated
        xTe = np.ascontiguousarray(xb.T.reshape(NCH, 128, S)).astype(np.float32)
        # maskT[p, kt*T + q] = mask[b, qt*T + q, k] with k = kt*128 + p in
        # ROTATED key order (keys follow the same rotation as tokens).
        msl = mask[b, qt * T:(qt + 1) * T][:, order]     # [T(q), S(k)] rotated
        maskTe = np.ascontiguousarray(
            msl.T.reshape(NKT, 128, T).transpose(1, 0, 2)
            .reshape(128, NKT * T)).astype(bf)
        in_maps.append({
            "xT": xTe, "maskT": maskTe,
            "wqT": wqTe, "wkT": wkTe, "wvT": wvTe, "woT": woTe,
            "w1T": w1Te, "w3T": w3Te, "w2T": w2Te,
            "ones16": ones16, "onesf": onesf,
        })
    return in_maps


_NC_CACHE = None


def get_nc():
    global _NC_CACHE
    if _NC_CACHE is None:
        _NC_CACHE = build_nc()
    return _NC_CACHE


def gather_output(results):
    out = np.empty((B, S, D), np.float32)
    for core in range(8):
        b, qt = core // NQT, core % NQT
        o = results[core]["outT"]              # [NCH, 128, T]
        out[b, qt * T:(qt + 1) * T, :] = o.reshape(D, T).T
    return out


def kernel(**inputs):
    from concourse.bass_utils import run_bass_kernel_spmd
    in_maps = prep_inputs(
        np.asarray(inputs["x"]), np.asarray(inputs["mask"]),
        np.asarray(inputs["wq"]), np.asarray(inputs["wk"]),
        np.asarray(inputs["wv"]), np.asarray(inputs["wo"]),
        np.asarray(inputs["w1"]), np.asarray(inputs["w2"]),
        np.asarray(inputs["w3"]),
        np.asarray(inputs["g_attn"]), np.asarray(inputs["g_ffn"]))
    nc = get_nc()
    res = run_bass_kernel_spmd(nc, in_maps, core_ids=list(range(8)))
    return gather_output(res.results)
